# revision 1
# baseline (speedup 1.0000x reference)
"""Trainium2 Bass kernel for the capsule-routing layer.

Math (derived from the reference):
  u_hat[b,i,j,k] = sum_d x[b,j,d] W[d, i*32+k]   (never materialized!)
  iter t: c = softmax_i(b_logits); s[i,k] = sum_j c[i,j] u_hat[i,j,k]
          o = s / sqrt(sum_k s^2 + eps); b_logits[i,j] = sum_k o[i,k] u_hat[i,j,k]
Substituting u_hat = x @ W everywhere:
  y[i,d]   = sum_j c[i,j] x[j,d]            (small matmul, K=1024)
  s[i,k]   = sum_d y[i,d] W[d, i*32+k]      (block-diagonal of y @ W)
  wtil[d,i]= sum_k W[d, i*32+k] o[i,k]      (W @ block-diag(o))
  b[i,j]   = sum_d x[j,d] wtil[d,i]         (small matmul, K=256)
This removes the 34-GFLOP u_hat product entirely (~7.6x FLOP reduction).

Sharding: data-parallel, 8 batches per core; batches processed in groups of
4 stacked on SBUF partitions (partition p = 32*b + i).
"""

import numpy as np

try:
    import concourse.bass as bass
except ImportError:  # path fallback for bare environments
    import sys

    sys.path.insert(0, "/opt/trn_rl_repo")
    import concourse.bass as bass

from contextlib import ExitStack

import concourse.bacc as bacc
import concourse.tile as tile
from concourse import mybir
from concourse.bass_utils import run_bass_kernel_spmd

F32 = mybir.dt.float32
F32R = mybir.dt.float32r
BF16 = mybir.dt.bfloat16
AF = mybir.ActivationFunctionType
ALU = mybir.AluOpType

NUM_CAPS = 32
DIM_CAPS = 32
D_IN = 256  # feature dim (d)
N_IN = 1024  # input capsule count (j)
IK = NUM_CAPS * DIM_CAPS  # 1024 flattened (i,k)
B_TOTAL = 64
N_CORES = 8
B_PER_CORE = 8
GB = 4  # batches per partition-group
GROUPS = B_PER_CORE // GB  # 2
EPS = 1e-7
ROUTINGS = 3
import os as _os
USE_DVE_RSQRT = _os.environ.get("K_DVE_RSQRT", "1") == "1"
PS_BUFS2 = _os.environ.get("K_PS_BUFS2", "1") == "1"
WORK_BUFS = int(_os.environ.get("K_WORK_BUFS", "3"))
X_BUFS = int(_os.environ.get("K_X_BUFS", "8"))
USE_TTR = _os.environ.get("K_TTR", "0") == "1"  # tensor_tensor_reduce crashes on HW 


def _r(ap):
    """Matmul operands are declared float32r; pass through."""
    return ap


def build_program():
    nc = bacc.Bacc("TRN2", target_bir_lowering=False, debug=False)

    x_b = nc.declare_dram_parameter("x_b", [B_PER_CORE, N_IN, D_IN], BF16, isOutput=False)
    x_d = nc.declare_dram_parameter("x_d", [B_PER_CORE, D_IN, N_IN], BF16, isOutput=False)
    w_a = nc.declare_dram_parameter("w_a", [D_IN, IK], F32R, isOutput=False)
    w_t = nc.declare_dram_parameter("w_t", [IK, D_IN], BF16, isOutput=False)
    mask_d = nc.declare_dram_parameter("mask", [128, IK], F32, isOutput=False)
    ident_d = nc.declare_dram_parameter("ident", [128, 128], F32, isOutput=False)
    out_d = nc.declare_dram_parameter("out", [GROUPS, 128, DIM_CAPS], F32, isOutput=True)

    with ExitStack() as ctx:
        tc = ctx.enter_context(tile.TileContext(nc))
        singles = ctx.enter_context(tc.tile_pool(name="singles", bufs=1))
        xpool = ctx.enter_context(tc.tile_pool(name="xpool", bufs=X_BUFS))
        work = ctx.enter_context(tc.tile_pool(name="work", bufs=WORK_BUFS))
        psum = ctx.enter_context(tc.tile_pool(name="ps", bufs=1, space="PSUM"))

        # ---- static tensors ----
        w_a_sb = singles.tile([128, 2, IK], F32R)  # [d%128, d//128, (ik)]
        nc.sync.dma_start(out=w_a_sb[:, :, :], in_=w_a[:, :].rearrange("(c p) n -> p c n", p=128))
        w_t_sb = singles.tile([128, 8, D_IN], BF16)  # [(ik)%128, (ik)//128, d]
        nc.sync.dma_start(out=w_t_sb[:, :, :], in_=w_t[:, :].rearrange("(c p) n -> p c n", p=128))
        mask_sb = singles.tile([128, IK], F32)
        nc.sync.dma_start(out=mask_sb[:, :], in_=mask_d[:, :])
        ident_sb = singles.tile([128, 128], F32)
        nc.sync.dma_start(out=ident_sb[:, :], in_=ident_d[:, :])
        cu_sb = singles.tile([128, NUM_CAPS], BF16)
        nc.vector.memset(cu_sb[:, :], 1.0 / NUM_CAPS)
        magic_sb = singles.tile([128, 1], mybir.dt.int32)
        nc.vector.memset(magic_sb[:, :], 0x5F3759DF)
        one_i_sb = singles.tile([128, 1], mybir.dt.int32)
        nc.vector.memset(one_i_sb[:, :], 1)

        def rsqrt_dve(a_ap, tagp):
            """1/sqrt(a): DVE quake bit-trick + Newton, or ACT Sqrt + recip."""
            if not USE_DVE_RSQRT:
                sn = work.tile([128, 1], F32, tag=tagp + "sn", name="nr_sn")
                nc.scalar.activation(sn[:, :], a_ap, AF.Sqrt)
                rr = work.tile([128, 1], F32, tag=tagp + "rr", name="nr_rr")
                nc.vector.reciprocal(rr[:, :], sn[:, :])
                return rr
            t_i = work.tile([128, 1], mybir.dt.int32, tag=tagp + "i", name="nr_i")
            nc.vector.tensor_tensor(
                t_i[:, :], a_ap.bitcast(mybir.dt.int32), one_i_sb[:, :], ALU.logical_shift_right
            )
            r = work.tile([128, 1], F32, tag=tagp + "r", name="nr_r")
            nc.vector.tensor_tensor(
                r[:, :].bitcast(mybir.dt.int32), magic_sb[:, :], t_i[:, :], ALU.subtract
            )
            t2 = work.tile([128, 1], F32, tag=tagp + "t", name="nr_t")
            for _ in range(3):
                nc.vector.tensor_mul(t2[:, :], a_ap, r[:, :])
                nc.vector.tensor_mul(t2[:, :], t2[:, :], r[:, :])
                nc.vector.tensor_scalar(t2[:, :], t2[:, :], -0.5, 1.5, ALU.mult, ALU.add)
                nc.vector.tensor_mul(r[:, :], r[:, :], t2[:, :])
            return r

        def group_stream(g):
            # ---- load this group's x in both layouts ----
            xb_t = []
            xd_t = []
            for b in range(GB):
                bb = g * GB + b
                xb = xpool.tile([128, 8, D_IN], BF16, tag="xb", name=f"xb{bb}")
                nc.sync.dma_start(out=xb[:, :, :], in_=x_b[bb].rearrange("(c p) n -> p c n", p=128))
                xb_t.append(xb)
                xd = xpool.tile([128, 2, N_IN], BF16, tag="xd", name=f"xd{bb}")
                nc.sync.dma_start(out=xd[:, :, :], in_=x_d[bb].rearrange("(c p) n -> p c n", p=128))
                xd_t.append(xd)
            yield

            cT_sb = None  # [j%128, j//128, (4b,32i)] softmax'd coupling coeffs
            for it in range(ROUTINGS):
                last = it == ROUTINGS - 1

                # ---- y-MM: y[b,i,d] = sum_j c[b,i,j] x[b,j,d] ----
                y4_ps = psum.tile([128, D_IN], F32, tag="m32", bufs=(2 if PS_BUFS2 else 1), name="y4_ps")
                for jc in range(8):
                    for b in range(GB):
                        lhsT = cu_sb[:, :] if it == 0 else cT_sb[:, jc, 32 * b : 32 * b + 32]
                        nc.tensor.matmul(
                            y4_ps[32 * b : 32 * b + 32, :],
                            _r(lhsT),
                            _r(xb_t[b][:, jc, :]),
                            start=(jc == 0),
                            stop=(jc == 7),
                            tile_position=(0, 32 * b),
                            skip_group_check=True,
                        )
                yield

                # evacuate + transpose y -> [d, (4b,32i)]
                y4_sb = work.tile([128, D_IN], F32, tag="y4sb", name="y4_sb")
                nc.scalar.copy(y4_sb[:, :], y4_ps[:, :])
                yT_ps = psum.tile([128, 2, 128], F32, tag="tp2", bufs=(2 if PS_BUFS2 else 1), name="yT_ps")
                for t in range(2):
                    nc.tensor.transpose(yT_ps[:, t, :], y4_sb[:, 128 * t : 128 * t + 128], ident_sb[:, :])
                yT_sb = work.tile([128, 2, 128], F32R, tag="yTsb", name="yT_sb")
                nc.vector.tensor_copy(yT_sb[:, :, :], yT_ps[:, :, :])
                yield

                # ---- s-MM (cross): s_cross[(b,i),(i'k)] = sum_d y[b,i,d] W[d,(i'k)] ----
                sc_ps = psum.tile([128, IK], F32, tag="big", bufs=2, name="sc_ps")
                if last:
                    m4_sb = work.tile([128, IK], F32, tag="m4f", name="m4f_sb")
                else:
                    m4_sb = work.tile([128, IK], BF16, tag="m4", name="m4_sb")
                for nh in range(2):
                    for dc in range(2):
                        nc.tensor.matmul(
                            sc_ps[:, 512 * nh : 512 * nh + 512],
                            _r(yT_sb[:, dc, :]),
                            _r(w_a_sb[:, dc, 512 * nh : 512 * nh + 512]),
                            start=(dc == 0),
                            stop=(dc == 1),
                            skip_group_check=True,
                        )
                    # mask this half (evacuates PSUM as it lands)
                    nc.vector.tensor_mul(
                        m4_sb[:, 512 * nh : 512 * nh + 512],
                        sc_ps[:, 512 * nh : 512 * nh + 512],
                        mask_sb[:, 512 * nh : 512 * nh + 512],
                    )
                yield

                if last:
                    # compact s[(b,i), k] = sum_i' masked[(b,i), (i',k)]
                    s4c = work.tile([128, DIM_CAPS], F32, tag="s4c", name="s4c")
                    nc.vector.tensor_reduce(
                        s4c[:, :],
                        m4_sb[:, :].rearrange("p (i k) -> p k i", i=NUM_CAPS),
                        axis=mybir.AxisListType.X,
                        op=ALU.add,
                    )
                    sq_s = work.tile([128, DIM_CAPS], F32, tag="sqs", name="sq_s")
                    nsq = work.tile([128, 1], F32, tag="nsq", name="nsq")
                    nc.scalar.activation(sq_s[:, :], s4c[:, :], AF.Square, accum_out=nsq[:, :])
                    nc.vector.tensor_scalar(nsq[:, :], nsq[:, :], EPS, None, ALU.add)
                    rn = rsqrt_dve(nsq[:, :], "lst")
                    o_out = work.tile([128, DIM_CAPS], F32, tag="oout", name="o_out")
                    nc.vector.tensor_scalar(o_out[:, :], s4c[:, :], rn[:, :], None, ALU.mult)
                    nc.sync.dma_start(out=out_d[g], in_=o_out[:, :])
                    return

                # ---- squash norm (sum of squares over free dim) ----
                sq_scr = work.tile([128, IK], BF16, tag="scr", name="sq_scr")
                nsq4 = work.tile([128, 1], F32, tag="nsq4", name="nsq4")
                nc.scalar.activation(sq_scr[:, :], m4_sb[:, :], AF.Square, accum_out=nsq4[:, :])
                nc.vector.tensor_scalar(nsq4[:, :], nsq4[:, :], EPS, None, ALU.add)
                rn4 = rsqrt_dve(nsq4[:, :], "mid")
                yield

                # ---- O = transpose(masked s) -> [(ik), (4b,32i)] via DMA xbar ----
                o_sb = work.tile([128, 8, 128], BF16, tag="osb", name="o_sb")
                for h in range(2):
                    nc.sync.dma_start_transpose(
                        o_sb[:, 4 * h : 4 * h + 4, :], m4_sb[:, 512 * h : 512 * h + 512]
                    )
                # ---- wtil-MM: wT[(b,i), d] = sum_(ik) O[(ik),(b,i)] WT[(ik), d] ----
                wT_ps = psum.tile([128, D_IN], F32, tag="m32", bufs=(2 if PS_BUFS2 else 1), name="wT_ps")
                for ikc in range(8):
                    for b in range(GB):
                        nc.tensor.matmul(
                            wT_ps[32 * b : 32 * b + 32, :],
                            _r(o_sb[:, ikc, 32 * b : 32 * b + 32]),
                            _r(w_t_sb[:, ikc, :]),
                            start=(ikc == 0),
                            stop=(ikc == 7),
                            tile_position=(0, 32 * b),
                            skip_group_check=True,
                        )
                yield

                # evacuate with the squash scale (o = s * rn) folded in
                wT_sb = work.tile([128, D_IN], BF16, tag="wTsb", name="wT_sb")
                nc.vector.tensor_scalar(wT_sb[:, :], wT_ps[:, :], rn4[:, :], None, ALU.mult)
                # transpose wtil -> [d, (4b,32i)] via DMA xbar
                wt_sb = work.tile([128, 2, 128], BF16, tag="wtsb", name="wt_sb")
                nc.sync.dma_start_transpose(wt_sb[:, :, :], wT_sb[:, :])
                yield

                # ---- b-MM: blogit[(b,i), j] = sum_d wtil[d,(b,i)] x[b][d, j] ----
                b4_ps = psum.tile([128, N_IN], F32, tag="big", bufs=2, name="b4_ps")
                e4_sb = work.tile([128, N_IN], BF16, tag="e4", name="e4_sb")
                eT_sb = work.tile([128, 8, 128], BF16, tag="eT", name="eT_sb")
                for jh in range(2):
                    for dc in range(2):
                        for b in range(GB):
                            nc.tensor.matmul(
                                b4_ps[32 * b : 32 * b + 32, 512 * jh : 512 * jh + 512],
                                _r(wt_sb[:, dc, 32 * b : 32 * b + 32]),
                                _r(xd_t[b][:, dc, 512 * jh : 512 * jh + 512]),
                                start=(dc == 0),
                                stop=(dc == 1),
                                tile_position=(0, 32 * b),
                                skip_group_check=True,
                            )
                    # softmax numerator + transpose for this half
                    nc.scalar.activation(
                        e4_sb[:, 512 * jh : 512 * jh + 512],
                        b4_ps[:, 512 * jh : 512 * jh + 512],
                        AF.Exp,
                    )
                    nc.sync.dma_start_transpose(
                        eT_sb[:, 4 * jh : 4 * jh + 4, :], e4_sb[:, 512 * jh : 512 * jh + 512]
                    )
                yield

                zT_sb = work.tile([128, 8, GB], F32, tag="zT", name="zT_sb")
                rz_sb = work.tile([128, 8, GB], F32, tag="rz", name="rz_sb")
                cT_sb = work.tile([128, 8, 128], BF16, tag="cT", name="cT_sb")
                for h in range(2):
                    hc = slice(4 * h, 4 * h + 4)
                    nc.vector.tensor_reduce(
                        zT_sb[:, hc, :],
                        eT_sb[:, hc, :].rearrange("p c (b i) -> p c b i", b=GB),
                        axis=mybir.AxisListType.X,
                        op=ALU.add,
                    )
                    nc.vector.reciprocal(rz_sb[:, hc, :], zT_sb[:, hc, :])
                    nc.vector.tensor_tensor(
                        cT_sb[:, hc, :].rearrange("p c (b i) -> p c b i", b=GB),
                        eT_sb[:, hc, :].rearrange("p c (b i) -> p c b i", b=GB),
                        rz_sb[:, hc, :].unsqueeze(3).broadcast_to([128, GB, GB, NUM_CAPS]),
                        ALU.mult,
                    )
                yield

        streams = [group_stream(g) for g in range(GROUPS)]
        alive = list(streams)
        while alive:
            keep = []
            for s in alive:
                try:
                    next(s)
                    keep.append(s)
                except StopIteration:
                    pass
            alive = keep

    nc.compile()
    return nc


def _host_inputs(x, W):
    import ml_dtypes

    bf16 = ml_dtypes.bfloat16
    x = np.ascontiguousarray(np.asarray(x, dtype=np.float32))
    W = np.ascontiguousarray(np.asarray(W, dtype=np.float32)).reshape(D_IN, IK)
    xT = np.ascontiguousarray(x.transpose(0, 2, 1)).astype(bf16)
    WT = np.ascontiguousarray(W.T).astype(bf16)
    x = x.astype(bf16)
    q = np.arange(IK)
    p = np.arange(128)
    mask = (q[None, :] // DIM_CAPS == p[:, None] % NUM_CAPS).astype(np.float32)
    ident = np.eye(128, dtype=np.float32)
    return x, xT, W, WT, mask, ident


_prog_cache = {}


def _get_program():
    if "nc" not in _prog_cache:
        _prog_cache["nc"] = build_program()
    return _prog_cache["nc"]


def kernel(x, W):
    x, xT, W, WT, mask, ident = _host_inputs(x, W)
    nc = _get_program()
    in_maps = []
    for c in range(N_CORES):
        sl = slice(c * B_PER_CORE, (c + 1) * B_PER_CORE)
        in_maps.append(
            {
                "x_b": x[sl],
                "x_d": xT[sl],
                "w_a": W,
                "w_t": WT,
                "mask": mask,
                "ident": ident,
            }
        )
    res = run_bass_kernel_spmd(nc, in_maps, core_ids=list(range(N_CORES)))
    out = np.empty((B_TOTAL, NUM_CAPS, DIM_CAPS), np.float32)
    for c in range(N_CORES):
        o = res.results[c]["out"]  # [GROUPS, 128, 32]; partition p = 32*b + i
        out[c * B_PER_CORE : (c + 1) * B_PER_CORE] = o.reshape(B_PER_CORE, NUM_CAPS, DIM_CAPS)
    return out



# revision 4
# speedup vs baseline: 1.0038x; 1.0038x over previous
"""Trainium2 Bass kernel for the capsule-routing layer.

Math (derived from the reference):
  u_hat[b,i,j,k] = sum_d x[b,j,d] W[d, i*32+k]   (never materialized!)
  iter t: c = softmax_i(b_logits); s[i,k] = sum_j c[i,j] u_hat[i,j,k]
          o = s / sqrt(sum_k s^2 + eps); b_logits[i,j] = sum_k o[i,k] u_hat[i,j,k]
Substituting u_hat = x @ W everywhere:
  y[i,d]    = sum_j c[i,j] x[j,d]           (small matmul, K=1024)
  s[i,k]    = sum_d y[i,d] W[d, i*32+k]     (block-diagonal of y @ W)
  wtil[d,i] = sum_k W[d, i*32+k] o[i,k]     (W @ block-diag(o))
  b[i,j]    = sum_d x[j,d] wtil[d,i]        (small matmul, K=256)
This removes the 34-GFLOP u_hat product entirely (~7.6x FLOP reduction).

Pipeline per routing iteration (partition p = 32*b + i, 4 batches/group):
  y-MM (col-tiled quadrants) -> PE-transpose -> s-MM (cross product vs all
  capsule blocks) -> mask -> O = m^T via DMA xbar -> wtil-MM with lhsT=W^T
  chunks giving wtil[d,(b,i)] directly (no second transpose) -> b-MM ->
  exp with the squash 1/|s| folded into the ACT per-partition scale ->
  e^T via DMA xbar -> softmax normalize.

Sharding: data-parallel, 8 batches per core, 2 groups of 4 interleaved.
DMA orchestration: input loads ride the SP HWDGE ring in an order that
unblocks compute ASAP; in-loop transposes + stores ride the ACT ring.
"""

import numpy as np

try:
    import concourse.bass as bass
except ImportError:  # path fallback for bare environments
    import sys

    sys.path.insert(0, "/opt/trn_rl_repo")
    import concourse.bass as bass

from contextlib import ExitStack

import concourse.bacc as bacc
import concourse.tile as tile
from concourse import mybir
from concourse.bass_utils import run_bass_kernel_spmd

F32 = mybir.dt.float32
BF16 = mybir.dt.bfloat16
AF = mybir.ActivationFunctionType
ALU = mybir.AluOpType

NUM_CAPS = 32
DIM_CAPS = 32
D_IN = 256  # feature dim (d)
N_IN = 1024  # input capsule count (j)
IK = NUM_CAPS * DIM_CAPS  # 1024 flattened (i,k)
B_TOTAL = 64
N_CORES = 8
B_PER_CORE = 8
GB = 4  # batches per partition-group
GROUPS = B_PER_CORE // GB  # 2
EPS = 1e-7
ROUTINGS = 3

import os as _os

WARMUP_MMS = int(_os.environ.get("K_WARMUP", "24"))
WORK_BUFS = int(_os.environ.get("K_WORK_BUFS", "3"))
X_BUFS = 8


def build_program():
    nc = bacc.Bacc("TRN2", target_bir_lowering=False, debug=False)

    x_b = nc.declare_dram_parameter("x_b", [B_PER_CORE, N_IN, D_IN], BF16, isOutput=False)
    x_d = nc.declare_dram_parameter("x_d", [B_PER_CORE, D_IN, N_IN], BF16, isOutput=False)
    w_a = nc.declare_dram_parameter("w_a", [D_IN, IK], BF16, isOutput=False)
    w_t = nc.declare_dram_parameter("w_t", [IK, D_IN], BF16, isOutput=False)
    mask_d = nc.declare_dram_parameter("mask", [128, IK], BF16, isOutput=False)
    ident_d = nc.declare_dram_parameter("ident", [128, 128], F32, isOutput=False)
    out_d = nc.declare_dram_parameter("out", [GROUPS, 128, DIM_CAPS], F32, isOutput=True)

    with ExitStack() as ctx:
        tc = ctx.enter_context(tile.TileContext(nc))
        singles = ctx.enter_context(tc.tile_pool(name="singles", bufs=1))
        xpool = ctx.enter_context(tc.tile_pool(name="xpool", bufs=X_BUFS))
        work = ctx.enter_context(tc.tile_pool(name="work", bufs=WORK_BUFS))
        psum = ctx.enter_context(tc.tile_pool(name="ps", bufs=1, space="PSUM"))

        # ---- identity first (tiny): unblocks PE warmup immediately ----
        ident_sb = singles.tile([128, 128], F32)
        nc.sync.dma_start(out=ident_sb[:, :], in_=ident_d[:, :])

        # ---- PE warmup: keep HAM un-throttled while inputs stream in ----
        warm_ps = psum.tile([128, 128], F32, tag="m32", bufs=2, name="warm_ps")
        for _ in range(WARMUP_MMS):
            nc.tensor.matmul(
                warm_ps[:, :], ident_sb[:, :], ident_sb[:, :],
                start=True, stop=True, skip_group_check=True,
            )

        # ---- input loads, in compute-unblock order (SP HWDGE ring) ----
        xb_t = [None] * B_PER_CORE
        xd_t = [None] * B_PER_CORE

        def load_xb(bb):
            t = xpool.tile([128, 8, D_IN], BF16, tag="xb", name=f"xb{bb}")
            nc.sync.dma_start(out=t[:, :, :], in_=x_b[bb].rearrange("(c p) n -> p c n", p=128))
            xb_t[bb] = t

        def load_xd(bb):
            t = xpool.tile([128, 2, N_IN], BF16, tag="xd", name=f"xd{bb}")
            nc.sync.dma_start(out=t[:, :, :], in_=x_d[bb].rearrange("(c p) n -> p c n", p=128))
            xd_t[bb] = t

        for bb in range(GB):  # group 0's y-MM inputs
            load_xb(bb)
        w_a_sb = singles.tile([128, 2, IK], BF16)  # [d%128, d//128, (ik)]
        nc.sync.dma_start(out=w_a_sb[:, :, :], in_=w_a[:, :].rearrange("(c p) n -> p c n", p=128))
        w_t_sb = singles.tile([128, 8, D_IN], BF16)  # [(ik)%128, (ik)//128, d]
        nc.sync.dma_start(out=w_t_sb[:, :, :], in_=w_t[:, :].rearrange("(c p) n -> p c n", p=128))
        mask_sb = singles.tile([128, IK], BF16)
        nc.sync.dma_start(out=mask_sb[:, :], in_=mask_d[:, :])
        for bb in range(GB):  # group 0's b-MM inputs
            load_xd(bb)
        for bb in range(GB, B_PER_CORE):  # group 1
            load_xb(bb)
        for bb in range(GB, B_PER_CORE):
            load_xd(bb)

        cu_sb = singles.tile([128, NUM_CAPS], BF16)
        nc.vector.memset(cu_sb[:, :], 1.0 / NUM_CAPS)
        eps_sb = singles.tile([128, 1], F32)
        nc.vector.memset(eps_sb[:, :], EPS)

        def group_stream(g):
            bs = [g * GB + b for b in range(GB)]
            yield

            cT_sb = None  # [j%128, j//128, (4b,32i)] softmax'd coupling coeffs
            rn4 = None  # [128, 1] squash 1/|s| for this iteration
            for it in range(ROUTINGS):
                last = it == ROUTINGS - 1

                # ---- y-MM: y[(b,i), d] = sum_j c[(b,i), j] x[b][j, d] ----
                y4_ps = psum.tile([128, D_IN], F32, tag="m32", bufs=2, name="y4_ps")
                for jc in range(8):
                    for b in range(GB):
                        lhsT = cu_sb[:, :] if it == 0 else cT_sb[:, jc, 32 * b : 32 * b + 32]
                        nc.tensor.matmul(
                            y4_ps[32 * b : 32 * b + 32, :],
                            lhsT,
                            xb_t[bs[b]][:, jc, :],
                            start=(jc == 0),
                            stop=(jc == 7),
                            tile_position=(0, 32 * b),
                            skip_group_check=True,
                        )
                yield

                # evacuate + PE-transpose y -> [d, (4b,32i)]
                y4_sb = work.tile([128, D_IN], F32, tag="y4sb", name="y4_sb")
                nc.scalar.copy(y4_sb[:, :], y4_ps[:, :])
                yT_ps = psum.tile([128, 2, 128], F32, tag="tp2", bufs=2, name="yT_ps")
                for t in range(2):
                    nc.tensor.transpose(yT_ps[:, t, :], y4_sb[:, 128 * t : 128 * t + 128], ident_sb[:, :])
                yT_sb = work.tile([128, 2, 128], BF16, tag="yTsb", name="yT_sb")
                nc.vector.tensor_copy(yT_sb[:, :, :], yT_ps[:, :, :])
                yield

                # ---- s-MM (cross): s[(b,i), (i'k)] = sum_d y[(b,i), d] W[d, (i'k)] ----
                sc_ps = psum.tile([128, IK], F32, tag="big", bufs=2, name="sc_ps")
                if last:
                    m4_sb = work.tile([128, IK], F32, tag="m4f", name="m4f_sb")
                else:
                    m4_sb = work.tile([128, IK], BF16, tag="m4", name="m4_sb")
                for nh in range(2):
                    for dc in range(2):
                        nc.tensor.matmul(
                            sc_ps[:, 512 * nh : 512 * nh + 512],
                            yT_sb[:, dc, :],
                            w_a_sb[:, dc, 512 * nh : 512 * nh + 512],
                            start=(dc == 0),
                            stop=(dc == 1),
                            skip_group_check=True,
                        )
                    # mask this half (evacuates PSUM as it lands)
                    nc.vector.tensor_mul(
                        m4_sb[:, 512 * nh : 512 * nh + 512],
                        sc_ps[:, 512 * nh : 512 * nh + 512],
                        mask_sb[:, 512 * nh : 512 * nh + 512],
                    )
                yield

                if last:
                    # compact s[(b,i), k] = sum_i' masked[(b,i), (i',k)]
                    s4c = work.tile([128, DIM_CAPS], F32, tag="s4c", name="s4c")
                    nc.vector.tensor_reduce(
                        s4c[:, :],
                        m4_sb[:, :].rearrange("p (i k) -> p k i", i=NUM_CAPS),
                        axis=mybir.AxisListType.X,
                        op=ALU.add,
                    )
                    sq_s = work.tile([128, DIM_CAPS], F32, tag="sqs", name="sq_s")
                    nsq = work.tile([128, 1], F32, tag="nsq", name="nsq")
                    nc.scalar.activation(sq_s[:, :], s4c[:, :], AF.Square, accum_out=nsq[:, :])
                    sn = work.tile([128, 1], F32, tag="sn", name="sn")
                    nc.scalar.activation(sn[:, :], nsq[:, :], AF.Sqrt, bias=eps_sb[:, :])
                    rn = work.tile([128, 1], F32, tag="rn", name="rn")
                    nc.vector.reciprocal(rn[:, :], sn[:, :])
                    o_out = work.tile([128, DIM_CAPS], F32, tag="oout", name="o_out")
                    nc.vector.tensor_scalar(o_out[:, :], s4c[:, :], rn[:, :], None, ALU.mult)
                    nc.scalar.dma_start(out=out_d[g], in_=o_out[:, :])
                    return

                # ---- squash norm: nsq = sum_k (masked s)^2 per partition ----
                sq_scr = work.tile([128, IK], BF16, tag="scr", name="sq_scr")
                nsq4 = work.tile([128, 1], F32, tag="nsq4", name="nsq4")
                nc.scalar.activation(sq_scr[:, :], m4_sb[:, :], AF.Square, accum_out=nsq4[:, :])
                sn4 = work.tile([128, 1], F32, tag="sn4", name="sn4")
                nc.scalar.activation(sn4[:, :], nsq4[:, :], AF.Sqrt, bias=eps_sb[:, :])
                rn4 = work.tile([128, 1], F32, tag="rn4", name="rn4")
                nc.vector.reciprocal(rn4[:, :], sn4[:, :])
                yield

                # ---- O = transpose(masked s) -> [(ik), (4b,32i)] via DMA xbar ----
                o_sb = work.tile([128, 8, 128], BF16, tag="osb", name="o_sb")
                for h in range(2):
                    nc.scalar.dma_start_transpose(
                        o_sb[:, 4 * h : 4 * h + 4, :], m4_sb[:, 512 * h : 512 * h + 512]
                    )
                # ---- wtil-MM: wtil[d, (b,i)] = sum_ik W^T[ik, d]^T... lhsT = w_t chunks ----
                wt_ps = psum.tile([128, 2, 128], F32, tag="tp2", bufs=2, name="wt_ps")
                for dc in range(2):
                    for ikc in range(8):
                        nc.tensor.matmul(
                            wt_ps[:, dc, :],
                            w_t_sb[:, ikc, 128 * dc : 128 * dc + 128],
                            o_sb[:, ikc, :],
                            start=(ikc == 0),
                            stop=(ikc == 7),
                            skip_group_check=True,
                        )
                wt_sb = work.tile([128, 2, 128], BF16, tag="wtsb", name="wt_sb")
                nc.vector.tensor_copy(wt_sb[:, :, :], wt_ps[:, :, :])
                yield

                # ---- b-MM: blogit[(b,i), j] = sum_d wtil[d, (b,i)] x[b][d, j] ----
                # exp evacuates with the squash scale folded in: e = exp(rn * b)
                b4_ps = psum.tile([128, N_IN], F32, tag="big", bufs=2, name="b4_ps")
                e4_sb = work.tile([128, N_IN], BF16, tag="e4", name="e4_sb")
                eT_sb = work.tile([128, 8, 128], BF16, tag="eT", name="eT_sb")
                for jh in range(2):
                    for dc in range(2):
                        for b in range(GB):
                            nc.tensor.matmul(
                                b4_ps[32 * b : 32 * b + 32, 512 * jh : 512 * jh + 512],
                                wt_sb[:, dc, 32 * b : 32 * b + 32],
                                xd_t[bs[b]][:, dc, 512 * jh : 512 * jh + 512],
                                start=(dc == 0),
                                stop=(dc == 1),
                                tile_position=(0, 32 * b),
                                skip_group_check=True,
                            )
                    nc.scalar.activation(
                        e4_sb[:, 512 * jh : 512 * jh + 512],
                        b4_ps[:, 512 * jh : 512 * jh + 512],
                        AF.Exp,
                        scale=rn4[:, :],
                    )
                    nc.scalar.dma_start_transpose(
                        eT_sb[:, 4 * jh : 4 * jh + 4, :], e4_sb[:, 512 * jh : 512 * jh + 512]
                    )
                yield

                # ---- softmax over i: z[j, b] = sum_i e^T; c^T = e^T / z ----
                zT_sb = work.tile([128, 8, GB], F32, tag="zT", name="zT_sb")
                rz_sb = work.tile([128, 8, GB], F32, tag="rz", name="rz_sb")
                cT_sb = work.tile([128, 8, 128], BF16, tag="cT", name="cT_sb")
                for h in range(2):
                    hc = slice(4 * h, 4 * h + 4)
                    nc.vector.tensor_reduce(
                        zT_sb[:, hc, :],
                        eT_sb[:, hc, :].rearrange("p c (b i) -> p c b i", b=GB),
                        axis=mybir.AxisListType.X,
                        op=ALU.add,
                    )
                    nc.vector.reciprocal(rz_sb[:, hc, :], zT_sb[:, hc, :])
                    nc.vector.tensor_tensor(
                        cT_sb[:, hc, :].rearrange("p c (b i) -> p c b i", b=GB),
                        eT_sb[:, hc, :].rearrange("p c (b i) -> p c b i", b=GB),
                        rz_sb[:, hc, :].unsqueeze(3).broadcast_to([128, GB, GB, NUM_CAPS]),
                        ALU.mult,
                    )
                yield

        streams = [group_stream(g) for g in range(GROUPS)]
        alive = list(streams)
        while alive:
            keep = []
            for s in alive:
                try:
                    next(s)
                    keep.append(s)
                except StopIteration:
                    pass
            alive = keep

    nc.compile()
    return nc


def _host_inputs(x, W):
    import ml_dtypes

    bf16 = ml_dtypes.bfloat16
    x = np.ascontiguousarray(np.asarray(x, dtype=np.float32))
    W = np.ascontiguousarray(np.asarray(W, dtype=np.float32)).reshape(D_IN, IK)
    xT = np.ascontiguousarray(x.transpose(0, 2, 1)).astype(bf16)
    WT = np.ascontiguousarray(W.T).astype(bf16)
    x = x.astype(bf16)
    W = W.astype(bf16)
    q = np.arange(IK)
    p = np.arange(128)
    mask = (q[None, :] // DIM_CAPS == p[:, None] % NUM_CAPS).astype(bf16)
    ident = np.eye(128, dtype=np.float32)
    return x, xT, W, WT, mask, ident


_prog_cache = {}


def _get_program():
    if "nc" not in _prog_cache:
        _prog_cache["nc"] = build_program()
    return _prog_cache["nc"]


def kernel(x, W):
    x, xT, W, WT, mask, ident = _host_inputs(x, W)
    nc = _get_program()
    in_maps = []
    for c in range(N_CORES):
        sl = slice(c * B_PER_CORE, (c + 1) * B_PER_CORE)
        in_maps.append(
            {
                "x_b": x[sl],
                "x_d": xT[sl],
                "w_a": W,
                "w_t": WT,
                "mask": mask,
                "ident": ident,
            }
        )
    res = run_bass_kernel_spmd(nc, in_maps, core_ids=list(range(N_CORES)))
    out = np.empty((B_TOTAL, NUM_CAPS, DIM_CAPS), np.float32)
    for c in range(N_CORES):
        o = res.results[c]["out"]  # [GROUPS, 128, 32]; partition p = 32*b + i
        out[c * B_PER_CORE : (c + 1) * B_PER_CORE] = o.reshape(B_PER_CORE, NUM_CAPS, DIM_CAPS)
    return out


# revision 6
# speedup vs baseline: 1.1646x; 1.1602x over previous
"""Trainium2 Bass kernel for the capsule-routing layer.

Math (derived from the reference):
  u_hat[b,i,j,k] = sum_d x[b,j,d] W[d, i*32+k]   (never materialized!)
  iter t: c = softmax_i(b_logits); s[i,k] = sum_j c[i,j] u_hat[i,j,k]
          o = s / sqrt(sum_k s^2 + eps); b_logits[i,j] = sum_k o[i,k] u_hat[i,j,k]
Substituting u_hat = x @ W everywhere:
  y[i,d]    = sum_j c[i,j] x[j,d]           (small matmul, K=1024)
  s[i,k]    = sum_d y[i,d] W[d, i*32+k]     (block-diagonal of y @ W)
  wtil[d,i] = sum_k W[d, i*32+k] o[i,k]     (W @ block-diag(o))
  b[i,j]    = sum_d x[j,d] wtil[d,i]        (small matmul, K=256)
This removes the 34-GFLOP u_hat product entirely (~7.6x FLOP reduction).

Pipeline per routing iteration (partition p = 32*b + i, 4 batches/group):
  y-MM (col-tiled quadrants) -> PE-transpose -> s-MM (cross product vs all
  capsule blocks) -> mask -> O = m^T via DMA xbar -> wtil-MM with lhsT=W^T
  chunks giving wtil[d,(b,i)] directly (no second transpose) -> b-MM ->
  exp with the squash 1/|s| folded into the ACT per-partition scale ->
  e^T via DMA xbar -> softmax normalize.

Sharding: data-parallel, 8 batches per core, 2 groups of 4 interleaved.
DMA orchestration: input loads ride the SP HWDGE ring in an order that
unblocks compute ASAP; in-loop transposes + stores ride the ACT ring.
"""

import numpy as np

try:
    import concourse.bass as bass
except ImportError:  # path fallback for bare environments
    import sys

    sys.path.insert(0, "/opt/trn_rl_repo")
    import concourse.bass as bass

from contextlib import ExitStack

import concourse.bacc as bacc
import concourse.tile as tile
from concourse import mybir
from concourse.bass_utils import run_bass_kernel_spmd

F32 = mybir.dt.float32
BF16 = mybir.dt.bfloat16
AF = mybir.ActivationFunctionType
ALU = mybir.AluOpType

NUM_CAPS = 32
DIM_CAPS = 32
D_IN = 256  # feature dim (d)
N_IN = 1024  # input capsule count (j)
IK = NUM_CAPS * DIM_CAPS  # 1024 flattened (i,k)
B_TOTAL = 64
N_CORES = 8
B_PER_CORE = 8
GB = 4  # batches per partition-group
GROUPS = B_PER_CORE // GB  # 2
EPS = 1e-7
ROUTINGS = 3

import os as _os

WARMUP_MMS = int(_os.environ.get("K_WARMUP", "24"))
WORK_BUFS = int(_os.environ.get("K_WORK_BUFS", "3"))
X_BUFS = 8


def build_program():
    nc = bacc.Bacc("TRN2", target_bir_lowering=False, debug=False)

    # x packed per 2-batch pair, pre-swizzled on host to [128, ...] partition tiles
    x_b = nc.declare_dram_parameter("x_b", [B_PER_CORE // 2, 128, 16, D_IN], BF16, isOutput=False)
    x_d = nc.declare_dram_parameter("x_d", [B_PER_CORE // 2, 128, 4, N_IN], BF16, isOutput=False)
    # W combo: [128, 2*IK (w_a) + 8*D_IN (w_t) + IK (mask)] bf16, host pre-swizzled
    w_c = nc.declare_dram_parameter("w_c", [128, 2 * IK + 8 * D_IN + IK], BF16, isOutput=False)
    ident_d = nc.declare_dram_parameter("ident", [128, 128], F32, isOutput=False)
    out_d = nc.declare_dram_parameter("out", [GROUPS, 128, DIM_CAPS], F32, isOutput=True)

    with ExitStack() as ctx:
        tc = ctx.enter_context(tile.TileContext(nc))
        singles = ctx.enter_context(tc.tile_pool(name="singles", bufs=1))
        xpool = ctx.enter_context(tc.tile_pool(name="xpool", bufs=X_BUFS))
        work = ctx.enter_context(tc.tile_pool(name="work", bufs=WORK_BUFS))
        psum = ctx.enter_context(tc.tile_pool(name="ps", bufs=1, space="PSUM"))

        # ---- identity first (tiny): unblocks PE warmup immediately ----
        ident_sb = singles.tile([128, 128], F32)
        nc.sync.dma_start(out=ident_sb[:, :], in_=ident_d[:, :])

        # ---- PE warmup: keep HAM un-throttled while inputs stream in ----
        warm_ps = psum.tile([128, 128], F32, tag="m32", bufs=2, name="warm_ps")
        for _ in range(WARMUP_MMS):
            nc.tensor.matmul(
                warm_ps[:, :], ident_sb[:, :], ident_sb[:, :],
                start=True, stop=True, skip_group_check=True,
            )

        # ---- input loads: few big DMAs in compute-unblock order (SP ring) ----
        xbp = [None] * (B_PER_CORE // 2)  # pair tiles [128, 16, D_IN]
        xdp = [None] * (B_PER_CORE // 2)  # pair tiles [128, 4, N_IN]

        def load_xb(pp):
            t = xpool.tile([128, 16, D_IN], BF16, tag="xb", name=f"xb{pp}")
            nc.sync.dma_start(out=t[:, :, :], in_=x_b[pp])
            xbp[pp] = t

        def load_xd(pp):
            t = xpool.tile([128, 4, N_IN], BF16, tag="xd", name=f"xd{pp}")
            nc.sync.dma_start(out=t[:, :, :], in_=x_d[pp])
            xdp[pp] = t

        load_xb(0)
        load_xb(1)
        w_c_sb = singles.tile([128, 2 * IK + 8 * D_IN + IK], BF16)
        nc.sync.dma_start(out=w_c_sb[:, :], in_=w_c[:, :])
        w_a_sb = w_c_sb[:, : 2 * IK].rearrange("p (c n) -> p c n", c=2)  # [d%128, d//128, ik]
        w_t_sb = w_c_sb[:, 2 * IK : 2 * IK + 8 * D_IN].rearrange("p (c n) -> p c n", c=8)
        mask_sb = w_c_sb[:, 2 * IK + 8 * D_IN :]
        load_xd(0)
        load_xd(1)
        load_xb(2)
        load_xb(3)
        load_xd(2)
        load_xd(3)

        def xb_ap(bb, jc):  # [128, 256] slice of batch bb, j-chunk jc
            return xbp[bb // 2][:, 8 * (bb % 2) + jc, :]

        def xd_ap(bb, dc, jh):  # [128, 512] slice of batch bb, d-chunk dc, j-half jh
            return xdp[bb // 2][:, 2 * (bb % 2) + dc, 512 * jh : 512 * jh + 512]

        cu_sb = singles.tile([128, NUM_CAPS], BF16)
        nc.vector.memset(cu_sb[:, :], 1.0 / NUM_CAPS)
        eps_sb = singles.tile([128, 1], F32)
        nc.vector.memset(eps_sb[:, :], EPS)

        def group_stream(g):
            bs = [g * GB + b for b in range(GB)]
            yield

            cT_sb = None  # [j%128, j//128, (4b,32i)] softmax'd coupling coeffs
            rn4 = None  # [128, 1] squash 1/|s| for this iteration
            for it in range(ROUTINGS):
                last = it == ROUTINGS - 1

                # ---- y-MM: y[(b,i), d] = sum_j c[(b,i), j] x[b][j, d] ----
                y4_ps = psum.tile([128, D_IN], F32, tag="m32", bufs=2, name="y4_ps")
                for jc in range(8):
                    for b in range(GB):
                        lhsT = cu_sb[:, :] if it == 0 else cT_sb[:, jc, 32 * b : 32 * b + 32]
                        nc.tensor.matmul(
                            y4_ps[32 * b : 32 * b + 32, :],
                            lhsT,
                            xb_ap(bs[b], jc),
                            start=(jc == 0),
                            stop=(jc == 7),
                            tile_position=(0, 32 * b),
                            skip_group_check=True,
                        )
                yield

                # evacuate + PE-transpose y -> [d, (4b,32i)]
                y4_sb = work.tile([128, D_IN], F32, tag="y4sb", name="y4_sb")
                nc.scalar.copy(y4_sb[:, :], y4_ps[:, :])
                yT_ps = psum.tile([128, 2, 128], F32, tag="tp2", bufs=2, name="yT_ps")
                for t in range(2):
                    nc.tensor.transpose(yT_ps[:, t, :], y4_sb[:, 128 * t : 128 * t + 128], ident_sb[:, :])
                yT_sb = work.tile([128, 2, 128], BF16, tag="yTsb", name="yT_sb")
                nc.vector.tensor_copy(yT_sb[:, :, :], yT_ps[:, :, :])
                yield

                # ---- s-MM (cross): s[(b,i), (i'k)] = sum_d y[(b,i), d] W[d, (i'k)] ----
                sc_ps = psum.tile([128, IK], F32, tag="big", bufs=2, name="sc_ps")
                if last:
                    m4_sb = work.tile([128, IK], F32, tag="m4f", name="m4f_sb")
                else:
                    m4_sb = work.tile([128, IK], BF16, tag="m4", name="m4_sb")
                for nh in range(2):
                    for dc in range(2):
                        nc.tensor.matmul(
                            sc_ps[:, 512 * nh : 512 * nh + 512],
                            yT_sb[:, dc, :],
                            w_a_sb[:, dc, 512 * nh : 512 * nh + 512],
                            start=(dc == 0),
                            stop=(dc == 1),
                            skip_group_check=True,
                        )
                    # mask this half (evacuates PSUM as it lands)
                    nc.vector.tensor_mul(
                        m4_sb[:, 512 * nh : 512 * nh + 512],
                        sc_ps[:, 512 * nh : 512 * nh + 512],
                        mask_sb[:, 512 * nh : 512 * nh + 512],
                    )
                yield

                if last:
                    # compact s[(b,i), k] = sum_i' masked[(b,i), (i',k)]
                    s4c = work.tile([128, DIM_CAPS], F32, tag="s4c", name="s4c")
                    nc.vector.tensor_reduce(
                        s4c[:, :],
                        m4_sb[:, :].rearrange("p (i k) -> p k i", i=NUM_CAPS),
                        axis=mybir.AxisListType.X,
                        op=ALU.add,
                    )
                    sq_s = work.tile([128, DIM_CAPS], F32, tag="sqs", name="sq_s")
                    nsq = work.tile([128, 1], F32, tag="nsq", name="nsq")
                    nc.scalar.activation(sq_s[:, :], s4c[:, :], AF.Square, accum_out=nsq[:, :])
                    sn = work.tile([128, 1], F32, tag="sn", name="sn")
                    nc.scalar.activation(sn[:, :], nsq[:, :], AF.Sqrt, bias=eps_sb[:, :])
                    rn = work.tile([128, 1], F32, tag="rn", name="rn")
                    nc.vector.reciprocal(rn[:, :], sn[:, :])
                    o_out = work.tile([128, DIM_CAPS], F32, tag="oout", name="o_out")
                    nc.vector.tensor_scalar(o_out[:, :], s4c[:, :], rn[:, :], None, ALU.mult)
                    nc.sync.dma_start(out=out_d[g], in_=o_out[:, :])
                    return

                # ---- squash norm: nsq = sum_k (masked s)^2 per partition ----
                sq_scr = work.tile([128, IK], BF16, tag="scr", name="sq_scr")
                nsq4 = work.tile([128, 1], F32, tag="nsq4", name="nsq4")
                nc.scalar.activation(sq_scr[:, :], m4_sb[:, :], AF.Square, accum_out=nsq4[:, :])
                sn4 = work.tile([128, 1], F32, tag="sn4", name="sn4")
                nc.scalar.activation(sn4[:, :], nsq4[:, :], AF.Sqrt, bias=eps_sb[:, :])
                rn4 = work.tile([128, 1], F32, tag="rn4", name="rn4")
                nc.vector.reciprocal(rn4[:, :], sn4[:, :])
                yield

                # ---- O = transpose(masked s) -> [(ik), (4b,32i)] via DMA xbar ----
                o_sb = work.tile([128, 8, 128], BF16, tag="osb", name="o_sb")
                for h in range(2):
                    nc.sync.dma_start_transpose(
                        o_sb[:, 4 * h : 4 * h + 4, :], m4_sb[:, 512 * h : 512 * h + 512]
                    )
                # ---- wtil-MM: wtil[d, (b,i)] = sum_ik W^T[ik, d]^T... lhsT = w_t chunks ----
                wt_ps = psum.tile([128, 2, 128], F32, tag="tp2", bufs=2, name="wt_ps")
                for dc in range(2):
                    for ikc in range(8):
                        nc.tensor.matmul(
                            wt_ps[:, dc, :],
                            w_t_sb[:, ikc, 128 * dc : 128 * dc + 128],
                            o_sb[:, ikc, :],
                            start=(ikc == 0),
                            stop=(ikc == 7),
                            skip_group_check=True,
                        )
                wt_sb = work.tile([128, 2, 128], BF16, tag="wtsb", name="wt_sb")
                nc.vector.tensor_copy(wt_sb[:, :, :], wt_ps[:, :, :])
                yield

                # ---- b-MM: blogit[(b,i), j] = sum_d wtil[d, (b,i)] x[b][d, j] ----
                # exp evacuates with the squash scale folded in: e = exp(rn * b)
                b4_ps = psum.tile([128, N_IN], F32, tag="big", bufs=2, name="b4_ps")
                e4_sb = work.tile([128, N_IN], BF16, tag="e4", name="e4_sb")
                eT_sb = work.tile([128, 8, 128], BF16, tag="eT", name="eT_sb")
                for jh in range(2):
                    for dc in range(2):
                        for b in range(GB):
                            nc.tensor.matmul(
                                b4_ps[32 * b : 32 * b + 32, 512 * jh : 512 * jh + 512],
                                wt_sb[:, dc, 32 * b : 32 * b + 32],
                                xd_ap(bs[b], dc, jh),
                                start=(dc == 0),
                                stop=(dc == 1),
                                tile_position=(0, 32 * b),
                                skip_group_check=True,
                            )
                    nc.scalar.activation(
                        e4_sb[:, 512 * jh : 512 * jh + 512],
                        b4_ps[:, 512 * jh : 512 * jh + 512],
                        AF.Exp,
                        scale=rn4[:, :],
                    )
                    nc.sync.dma_start_transpose(
                        eT_sb[:, 4 * jh : 4 * jh + 4, :], e4_sb[:, 512 * jh : 512 * jh + 512]
                    )
                yield

                # ---- softmax over i: z[j, b] = sum_i e^T; c^T = e^T / z ----
                zT_sb = work.tile([128, 8, GB], F32, tag="zT", name="zT_sb")
                rz_sb = work.tile([128, 8, GB], F32, tag="rz", name="rz_sb")
                cT_sb = work.tile([128, 8, 128], BF16, tag="cT", name="cT_sb")
                for h in range(2):
                    hc = slice(4 * h, 4 * h + 4)
                    nc.vector.tensor_reduce(
                        zT_sb[:, hc, :],
                        eT_sb[:, hc, :].rearrange("p c (b i) -> p c b i", b=GB),
                        axis=mybir.AxisListType.X,
                        op=ALU.add,
                    )
                    nc.vector.reciprocal(rz_sb[:, hc, :], zT_sb[:, hc, :])
                    nc.vector.tensor_tensor(
                        cT_sb[:, hc, :].rearrange("p c (b i) -> p c b i", b=GB),
                        eT_sb[:, hc, :].rearrange("p c (b i) -> p c b i", b=GB),
                        rz_sb[:, hc, :].unsqueeze(3).broadcast_to([128, GB, GB, NUM_CAPS]),
                        ALU.mult,
                    )
                yield

        streams = [group_stream(g) for g in range(GROUPS)]
        alive = list(streams)
        while alive:
            keep = []
            for s in alive:
                try:
                    next(s)
                    keep.append(s)
                except StopIteration:
                    pass
            alive = keep

    nc.compile()
    return nc


def make_in_maps(x, W):
    import ml_dtypes

    bf16 = ml_dtypes.bfloat16
    x = np.ascontiguousarray(np.asarray(x, dtype=np.float32))
    W = np.ascontiguousarray(np.asarray(W, dtype=np.float32)).reshape(D_IN, IK)
    xT = x.transpose(0, 2, 1)  # [B, d, j]
    # x_b pairs: [B/2, 128(jp), (b2, jc)=16, d]
    xb = np.ascontiguousarray(
        x.reshape(B_TOTAL // 2, 2, 8, 128, D_IN).transpose(0, 3, 1, 2, 4)
        .reshape(B_TOTAL // 2, 128, 16, D_IN)
    ).astype(bf16)
    # x_d pairs: [B/2, 128(dp), (b2, dc)=4, j]
    xd = np.ascontiguousarray(
        xT.reshape(B_TOTAL // 2, 2, 2, 128, N_IN).transpose(0, 3, 1, 2, 4)
        .reshape(B_TOTAL // 2, 128, 4, N_IN)
    ).astype(bf16)
    # W combo: [128, 2*IK | 8*D_IN | IK]
    wa = W.reshape(2, 128, IK).transpose(1, 0, 2).reshape(128, 2 * IK)
    wt = W.T.reshape(8, 128, D_IN).transpose(1, 0, 2).reshape(128, 8 * D_IN)
    q = np.arange(IK)
    p = np.arange(128)
    mask = (q[None, :] // DIM_CAPS == p[:, None] % NUM_CAPS).astype(np.float32)
    w_c = np.ascontiguousarray(np.concatenate([wa, wt, mask], axis=1)).astype(bf16)
    ident = np.eye(128, dtype=np.float32)
    in_maps = []
    pairs_per_core = B_PER_CORE // 2
    for c in range(N_CORES):
        sl = slice(c * pairs_per_core, (c + 1) * pairs_per_core)
        in_maps.append({"x_b": xb[sl], "x_d": xd[sl], "w_c": w_c, "ident": ident})
    return in_maps


_prog_cache = {}


def _get_program():
    if "nc" not in _prog_cache:
        _prog_cache["nc"] = build_program()
    return _prog_cache["nc"]


def kernel(x, W):
    in_maps = make_in_maps(x, W)
    nc = _get_program()
    res = run_bass_kernel_spmd(nc, in_maps, core_ids=list(range(N_CORES)))
    out = np.empty((B_TOTAL, NUM_CAPS, DIM_CAPS), np.float32)
    for c in range(N_CORES):
        o = res.results[c]["out"]  # [GROUPS, 128, 32]; partition p = 32*b + i
        out[c * B_PER_CORE : (c + 1) * B_PER_CORE] = o.reshape(B_PER_CORE, NUM_CAPS, DIM_CAPS)
    return out
